# revision 41
# baseline (speedup 1.0000x reference)
"""Multi-head attention (B=2, S=2048, D=768, H=12) on 8 TRN2 NeuronCores.

Sharding: core c -> batch b = c//4, head-group g = c%4 (3 heads of 64 each).

v2 design (vs v1 baseline):
  - Softmax pipeline: E^T matmuls (K=64, h0/h1 row-group paired at partitions
    0-63/64-127; h2 paired with itself via duplicated k/q rows) -> scalar-engine
    exp (the only exp-capable engine; it paces the kernel at ~96us) -> 0/1 mask
    multiply in bf16 SBUF split across DVE (h0/h1) and GpSimd (h2) -> PV with a
    ones column in v for the denominator row.
  - v projected directly into [seq, d] layout (x-stationary matmuls), killing
    the 32 PE transposes of v1.
  - Output projection stacks h0/h1 into one K=128 matmul; h2 (K=64) pairs
    across q-tile parity via duplicated onorm2/wo2 rows.
  - Denominator reciprocal via reciprocal_approx_fast (0.7us vs 4us each).
  - Issue-order software pipeline: background projection / PV work drained
    between E columns to keep PE warm and dense.
"""

import os
import sys

sys.path.insert(0, "/opt/trn_rl_repo")

from collections import deque
from contextlib import ExitStack

import ml_dtypes
import numpy as np

import concourse.bass as bass
import concourse.mybir as mybir
import concourse.tile as tile
from concourse import bacc
from concourse.bass import ds
from concourse.bass_utils import run_bass_kernel_spmd

F32 = mybir.dt.float32
BF16 = mybir.dt.bfloat16

SEQ = 2048
D = 768
HD = 64
GD = 192          # head-group width = 3 heads * 64
QB = 512          # q-block
NQB = SEQ // QB   # 4
KT = SEQ // 128   # 16 k-tiles
NCH = 8           # chunks per (h, n): 2 k-tiles each
SCALE = float(1.0 / np.sqrt(np.float32(D)))

_CACHE = {}
DBG_EXACT_RECIP = bool(int(os.environ.get("DBG_EXACT_RECIP", "0")))
DBG_NO_H2DUP = bool(int(os.environ.get("DBG_NO_H2DUP", "0")))
DBG_ALL_VEC_MASK = bool(int(os.environ.get("DBG_ALL_VEC_MASK", "0")))
DBG_NO_COLPAIR = bool(int(os.environ.get("DBG_NO_COLPAIR", "0")))


def _install_profile_hook():
    import types

    if "antenv.axon_hooks" in sys.modules:
        return
    sys.path.insert(0, "/root/.axon_site")
    try:
        from trn_agent_boot.trn_boot import _ntff_profile_via_ctypes
        hook = _ntff_profile_via_ctypes("/opt/axon/libaxon_pjrt.so")
    except Exception:
        hook = None
    import concourse.bass_utils as _bu

    _bu.upload_artifacts = lambda tmpdir: tmpdir
    mod = types.ModuleType("antenv.axon_hooks")
    mod.get_axon_ntff_profile_hook = lambda: hook
    mod.set_axon_ntff_profile_hook = lambda h: None
    sys.modules["antenv.axon_hooks"] = mod


def _build():
    nc = bacc.Bacc(None)

    xqT = nc.declare_dram_parameter("xqT", [D, SEQ], BF16, isOutput=False)
    xkT = nc.declare_dram_parameter("xkT", [D, SEQ], BF16, isOutput=False)
    xvT = nc.declare_dram_parameter("xvT", [D, SEQ], BF16, isOutput=False)
    wqT = nc.declare_dram_parameter("wqT", [D, GD], BF16, isOutput=False)
    wkT = nc.declare_dram_parameter("wkT", [D, GD], BF16, isOutput=False)
    wvT = nc.declare_dram_parameter("wvT", [D, GD], BF16, isOutput=False)
    woT = nc.declare_dram_parameter("woT", [GD, D], BF16, isOutput=False)
    maskT = nc.declare_dram_parameter("maskT", [SEQ, SEQ], BF16, isOutput=False)
    out = nc.declare_dram_parameter("out", [SEQ, D], F32, isOutput=True)
    DBG_DUMP = bool(int(os.environ.get("DBG_DUMP", "0")))
    if DBG_DUMP:
        dbg = {}
        for nm in ("qA", "kA", "qB", "kB", "onormA", "onorm2"):
            dbg[nm] = nc.declare_dram_parameter(f"dbg_{nm}", [128, SEQ], BF16, isOutput=True)
        dbg["vall"] = nc.declare_dram_parameter("dbg_vall", [128, KT * 3 * 66], BF16, isOutput=True)
        dbg["P0"] = nc.declare_dram_parameter("dbg_P0", [128, KT * QB], BF16, isOutput=True)
        dbg["ou"] = nc.declare_dram_parameter("dbg_ou", [128, QB], F32, isOutput=True)

    with tile.TileContext(nc) as tc, ExitStack() as ctx:
        Exp = mybir.ActivationFunctionType.Exp

        # ---- persistent SBUF ------------------------------------------------
        pp = ctx.enter_context(tc.tile_pool(name="persist", bufs=1))
        qA = pp.tile([128, SEQ], BF16, tag="qA")    # h0 p0-63, h1 p64-127
        kA = pp.tile([128, SEQ], BF16, tag="kA")
        qB = pp.tile([128, SEQ], BF16, tag="qB")    # h2 duplicated both halves
        kB = pp.tile([128, SEQ], BF16, tag="kB")
        # v with ones column at 64 (66-wide for 4B alignment); [k, kt, h, 66]
        v_all = pp.tile([128, KT, 3, 66], BF16, tag="vall")
        onormA = pp.tile([128, SEQ], BF16, tag="onormA")  # h0 p0-63, h1 p64-127
        onorm2 = pp.tile([128, SEQ], BF16, tag="onorm2")  # h2 duplicated
        woA_sb = pp.tile([128, D], BF16, tag="woA")
        wo2_sb = pp.tile([128, D], BF16, tag="wo2")       # h2 rows duplicated
        w_sb = {}
        for name, wT in (("q", wqT), ("k", wkT), ("v", wvT)):
            w_sb[name] = pp.tile([128, 6, GD], BF16, tag=f"w{name}", name=f"w_{name}")
            nc.sync.dma_start(
                w_sb[name][:], wT.rearrange("(ko ki) d -> ki ko d", ki=128)
            )
        nc.sync.dma_start(woA_sb[:], woT[ds(0, 128), :])
        nc.sync.dma_start(wo2_sb[0:64, :], woT[ds(128, 64), :])
        nc.sync.dma_start(wo2_sb[64:128, :], woT[ds(128, 64), :])
        nc.vector.memset(v_all[:, :, :, 64:65], 1.0)

        # ---- x / mask staging ----------------------------------------------
        xvp = ctx.enter_context(tc.tile_pool(name="xv", bufs=1))
        xqp = ctx.enter_context(tc.tile_pool(name="xq", bufs=4))
        mp = ctx.enter_context(tc.tile_pool(name="mask", bufs=2))

        xv_sb = xvp.tile([128, 6, SEQ], BF16, tag="xv")
        xkp_stack = ExitStack()
        xkp = xkp_stack.enter_context(tc.tile_pool(name="xk", bufs=1))
        xk_sb = xkp.tile([128, 6, SEQ], BF16, tag="xk")
        xkTr = xkT.rearrange("(ko ki) s -> ki ko s", ki=128)
        xvTr = xvT.rearrange("(ko ki) s -> ki ko s", ki=128)
        xqTr = xqT.rearrange("(ko ki) s -> ki ko s", ki=128)
        maskTr = maskT.rearrange("(ko ki) q -> ki ko q", ki=128)
        for k in range(6):
            nc.sync.dma_start(xk_sb[:, k, :], xkTr[:, k, :])
        for k2 in range(3):
            nc.sync.dma_start(
                xv_sb[:, ds(k2 * 2, 2), :], xvTr[:, ds(k2 * 2, 2), :]
            )

        def dma_xq(n):
            t = xqp.tile([128, 6, QB], BF16, tag="xq", name=f"xq{n}")
            nc.sync.dma_start(t[:], xqTr[:, :, ds(n * QB, QB)])
            return t

        def dma_mask(n, half):
            t = mp.tile([128, NCH, QB], BF16, tag="m", name=f"m{n}_{half}")
            nc.sync.dma_start(
                t[:], maskTr[:, ds(half * NCH, NCH), ds(n * QB, QB)]
            )
            return t

        xq_t = {0: dma_xq(0)}
        mask_t = {(0, 0): dma_mask(0, 0), (0, 1): dma_mask(0, 1)}

        # ---- preamble projections: kA, B(n0), qA(n0) -----------------------
        pre_ps = ExitStack()
        pj = pre_ps.enter_context(tc.tile_pool(name="pj", bufs=4, space="PSUM"))
        psB = pre_ps.enter_context(tc.tile_pool(name="psB", bufs=1, space="PSUM"))

        def proj_A(name, x_ap, dst, n):
            """[128,512] psum accumulation of heads 0/1 for q-block n."""
            t = pj.tile([128, QB], F32, tag="pj", name=f"pj_{name}{n}")
            for k in range(6):
                nc.tensor.matmul(
                    t[:], lhsT=w_sb[name][:, k, ds(0, 128)],
                    rhs=x_ap[k], start=(k == 0), stop=(k == 5),
                )
            nc.vector.tensor_copy(dst[:, ds(n * QB, QB)], t[:])

        def proj_B(x_q, x_k, n, psum_tile=None):
            """col-paired h2 proj of q (rows 0-63) and k (rows 64-127)."""
            t = psum_tile if psum_tile is not None else psB.tile(
                [128, QB], F32, tag="psB", name=f"psB{n}"
            )
            if DBG_NO_COLPAIR:
                for k in range(6):
                    nc.tensor.matmul(
                        t[0:64, :], lhsT=w_sb["q"][:, k, ds(128, 64)],
                        rhs=x_q[k], start=(k == 0), stop=(k == 5),
                    )
                for k in range(6):
                    nc.tensor.matmul(
                        t[64:128, :], lhsT=w_sb["k"][:, k, ds(128, 64)],
                        rhs=x_k[k], start=(k == 0), stop=(k == 5),
                        tile_position=(0, 64),
                    )
            else:
                for k in range(6):
                    nc.tensor.matmul(
                        t[0:64, :], lhsT=w_sb["q"][:, k, ds(128, 64)],
                        rhs=x_q[k], start=(k == 0), stop=(k == 5),
                        tile_position=(0, 0),
                    )
                    nc.tensor.matmul(
                        t[64:128, :], lhsT=w_sb["k"][:, k, ds(128, 64)],
                        rhs=x_k[k], start=(k == 0), stop=(k == 5),
                        tile_position=(0, 64),
                    )
            sl = ds(n * QB, QB)
            nc.vector.tensor_copy(qB[0:64, sl], t[0:64, :])
            nc.vector.tensor_copy(qB[64:128, sl], t[0:64, :])
            nc.vector.tensor_copy(kB[0:64, sl], t[64:128, :])
            nc.vector.tensor_copy(kB[64:128, sl], t[64:128, :])

        # kA first (gates all E matmuls), then h2 B-pair n0, then qA n0
        kA_ps = [pj.tile([128, QB], F32, tag="pj", name=f"pj_k{n}") for n in range(NQB)]
        for k in range(6):
            for n in range(NQB):
                nc.tensor.matmul(
                    kA_ps[n][:], lhsT=w_sb["k"][:, k, ds(0, 128)],
                    rhs=xk_sb[:, k, ds(n * QB, QB)],
                    start=(k == 0), stop=(k == 5),
                )
        for n in range(NQB):
            nc.vector.tensor_copy(kA[:, ds(n * QB, QB)], kA_ps[n][:])
        for bn in range(NQB):
            if bn > 0 and bn not in xq_t:
                xq_t[bn] = dma_xq(bn)
            proj_B([xq_t[bn][:, k, :] for k in range(6)],
                   [xk_sb[:, k, ds(bn * QB, QB)] for k in range(6)], bn)
        proj_A("q", [xq_t[0][:, k, :] for k in range(6)], qA, 0)
        pre_ps.close()
        xkp_stack.close()

        # ---- attention pipeline --------------------------------------------
        att_ps = ExitStack()
        ou_ps = att_ps.enter_context(tc.tile_pool(name="ou", bufs=2, space="PSUM"))
        e_ps = att_ps.enter_context(tc.tile_pool(name="e", bufs=3, space="PSUM"))
        pp2 = ctx.enter_context(tc.tile_pool(name="P", bufs=3 if DBG_DUMP else 4))
        rp = ctx.enter_context(tc.tile_pool(name="rp", bufs=1))
        op = ctx.enter_context(tc.tile_pool(name="op", bufs=2))

        filler = deque()
        if DBG_DUMP:
            dbg_P0_sb = pp.tile([128, KT * QB], BF16, tag="dbgP0")
            dbg_ou_sb = pp.tile([128, QB], F32, tag="dbgou")

        def v_pair_task(s):
            def run():
                t = ou_ps.tile([128, GD], F32, tag="ou", name=f"vps{s}")
                for k in range(6):
                    nc.tensor.matmul(
                        t[:], lhsT=xv_sb[:, k, ds(s * 128, 128)],
                        rhs=w_sb["v"][:, k, :],
                        start=(k == 0), stop=(k == 5),
                    )
                nc.vector.tensor_copy(v_all[:, s, :, 0:64], t[:])
            return run

        def qA_task(n):
            def run():
                t = e_ps.tile([128, 2, QB], F32, tag="e", name=f"qa{n}")
                for k in range(6):
                    nc.tensor.matmul(
                        t[:, 0, :], lhsT=w_sb["q"][:, k, ds(0, 128)],
                        rhs=xq_t[n][:, k, :], start=(k == 0), stop=(k == 5),
                    )
                nc.vector.tensor_copy(qA[:, ds(n * QB, QB)], t[:, 0, :])
            return run

        def qB_task(n):
            def run():
                t = e_ps.tile([128, 2, QB], F32, tag="e", name=f"qb{n}")
                proj_B([xq_t[n][:, k, :] for k in range(6)],
                       [xk_sb[:, k, ds(n * QB, QB)] for k in range(6)],
                       n, psum_tile=t[:, 0, :])
            return run

        def pv_task(h, n, P_t, grp):
            def run():
                if grp == 0:
                    pv_task.ou[(h, n)] = ou_ps.tile(
                        [HD + 1, QB], F32, tag="ou", name=f"ou{h}_{n}"
                    )
                t = pv_task.ou[(h, n)]
                for j in range(4):
                    m = grp * 4 + j
                    nc.tensor.matmul(
                        t[:], lhsT=v_all[:, m, h, 0:65],
                        rhs=P_t[:, m, :],
                        start=(m == 0), stop=(m == KT - 1),
                    )
                if grp == 3:
                    # normalize: recip of denominator row, broadcast, multiply
                    if DBG_DUMP and h == 2 and n == 0:
                        nc.vector.tensor_copy(dbg_ou_sb[0:65, :], t[:])
                    sl = ds(n * QB, QB)
                    d_sb = rp.tile([1, QB], F32, tag="dc", name=f"dc{h}_{n}")
                    nc.vector.tensor_copy(d_sb[:], t[HD : HD + 1, :])
                    r1 = rp.tile([1, QB], F32, tag="r1", name=f"r1{h}_{n}")
                    if DBG_EXACT_RECIP:
                        nc.vector.reciprocal(r1[:], d_sb[:])
                    else:
                        nc.vector.reciprocal_approx_fast(r1[:], d_sb[:])
                    rb = rp.tile([64, QB], F32, tag="rb", name=f"rb{h}_{n}")
                    nc.gpsimd.partition_broadcast(rb[:], r1[:])
                    if DBG_DUMP and h == 2 and n == 0:
                        nc.vector.tensor_copy(dbg_ou_sb[96:97, :], r1[:])
                    if h == 0:
                        nc.vector.tensor_mul(onormA[0:64, sl], t[0:HD, :], rb[:])
                    elif h == 1:
                        nc.vector.tensor_mul(onormA[64:128, sl], t[0:HD, :], rb[:])
                    else:
                        nc.vector.tensor_mul(onorm2[0:64, sl], t[0:HD, :], rb[:])
                        nc.vector.tensor_mul(onorm2[64:128, sl], t[0:HD, :], rb[:])
            return run

        pv_task.ou = {}

        def oproj_task(mq):
            def run():
                p0 = 0 if mq % 2 == 0 else 64
                o_sb = op.tile([128, D], F32, tag="o", name=f"o{mq}")
                for j, (n0, nw) in enumerate(((0, 512), (512, 256))):
                    fp = ou_ps.tile([128, nw], F32, tag="ou", name=f"f{mq}_{j}")
                    nc.tensor.matmul(
                        fp[:], lhsT=onormA[:, ds(mq * 128, 128)],
                        rhs=woA_sb[:, ds(n0, nw)], start=True, stop=False,
                    )
                    nc.tensor.matmul(
                        fp[:], lhsT=onorm2[p0 : p0 + 64, ds(mq * 128, 128)],
                        rhs=wo2_sb[p0 : p0 + 64, ds(n0, nw)],
                        start=False, stop=True,
                    )
                    eng = nc.scalar if mq % 2 == 0 else nc.vector
                    if mq % 2 == 0:
                        nc.scalar.copy(o_sb[:, ds(n0, nw)], fp[:])
                    else:
                        nc.vector.tensor_copy(o_sb[:, ds(n0, nw)], fp[:])
                d_eng = (nc.sync, nc.gpsimd)[mq % 2]
                d_eng.dma_start(out[ds(mq * 128, 128), :], o_sb[:])
            return run

        def drain(k):
            for _ in range(k):
                if filler:
                    filler.popleft()()

        for s in range(KT):
            filler.append(v_pair_task(s))

        q_of = (qA, qA, qB)
        k_of = (kA, kA, kB)
        pbase = (0, 64, 0)

        for n in range(NQB):

            P = [pp2.tile([128, KT, QB], BF16, tag="P", name=f"P{n}_{h}")
                 for h in range(3)]
            if DBG_DUMP and n == 1:
                nc.vector.tensor_copy(dbg_P0_sb[:], P_prev0[:])
            if n == 0:
                P_prev0 = P[2]
            nsl = ds(n * QB, QB)
            for c in range(NCH):
                m0, m1 = 2 * c, 2 * c + 1
                half, hsl = c // 4, ds((c % 4) * 2, 2)
                mk = mask_t[(n, half)]
                for h in range(3):
                    e = e_ps.tile([128, 2, QB], F32, tag="e", name=f"e{n}_{c}_{h}")
                    for mm, m in ((0, m0), (1, m1)):
                        if h == 2:
                            p0 = 0 if (mm == 0 or DBG_NO_H2DUP) else 64
                        else:
                            p0 = pbase[h]
                        nc.tensor.matmul(
                            e[:, mm, :],
                            lhsT=k_of[h][p0 : p0 + 64, ds(m * 128, 128)],
                            rhs=q_of[h][p0 : p0 + 64, nsl],
                            start=True, stop=True,
                        )
                    if DBG_DUMP and n == 0 and c == 4 and h == 2:
                        nc.vector.tensor_copy(dbg_ou_sb[0:128, :], e[:, 0, :])
                    nc.scalar.activation(
                        P[h][:, ds(2 * c, 2), :], e[:, :, :], Exp,
                        scale=SCALE,
                    )
                # batched mask multiplies: DVE for h0/h1 every 2 chunks,
                # gpsimd for h2 once per mask half (big op, fewer dispatches)
                if c % 2 == 1:
                    for h in (0, 1):
                        nc.vector.tensor_mul(
                            P[h][:, ds((c - 1) * 2, 4), :],
                            P[h][:, ds((c - 1) * 2, 4), :],
                            mk[:, ds(((c - 1) % 4) * 2, 4), :],
                        )
                if c % 4 == 3:
                    eng2 = nc.vector if DBG_ALL_VEC_MASK else nc.gpsimd
                    eng2.tensor_mul(
                        P[2][:, ds((c - 3) * 2, 8), :],
                        P[2][:, ds((c - 3) * 2, 8), :],
                        mk[:, :, :],
                    )
                drain(3)
                if c == 3 and n + 1 < NQB:
                    mask_t[(n + 1, 0)] = dma_mask(n + 1, 0)
                if c == 5 and n + 1 < NQB:
                    qA_task(n + 1)()
            if n + 1 < NQB:
                mask_t[(n + 1, 1)] = dma_mask(n + 1, 1)
            for h in range(3):
                for g in range(4):
                    filler.append(pv_task(h, n, P[h], g))
            for mq in range(4 * n, 4 * n + 4):
                filler.append(oproj_task(mq))
            drain(8)

        while filler:
            drain(1)
        att_ps.close()
    nc.compile()
    return nc


def kernel(Q, K, V, mask, Wq, Wk, Wv, Wo):
    if "nc" not in _CACHE:
        _CACHE["nc"] = _build()
    nc = _CACHE["nc"]

    maskT_bf = np.ascontiguousarray(
        (mask[0, 0].T != 0).astype(ml_dtypes.bfloat16)
    )
    in_maps = []
    for c in range(8):
        b, g = c // 4, c % 4
        sl = slice(g * GD, (g + 1) * GD)
        in_maps.append(
            {
                "xqT": np.ascontiguousarray(Q[b].T.astype(ml_dtypes.bfloat16)),
                "xkT": np.ascontiguousarray(K[b].T.astype(ml_dtypes.bfloat16)),
                "xvT": np.ascontiguousarray(V[b].T.astype(ml_dtypes.bfloat16)),
                "wqT": np.ascontiguousarray(Wq[sl, :].T.astype(ml_dtypes.bfloat16)),
                "wkT": np.ascontiguousarray(Wk[sl, :].T.astype(ml_dtypes.bfloat16)),
                "wvT": np.ascontiguousarray(Wv[sl, :].T.astype(ml_dtypes.bfloat16)),
                "woT": np.ascontiguousarray(Wo[:, sl].T.astype(ml_dtypes.bfloat16)),
                "maskT": maskT_bf,
            }
        )

    _install_profile_hook()
    res = run_bass_kernel_spmd(
        nc,
        in_maps,
        core_ids=list(range(8)),
        trace=bool(int(os.environ.get("KERNEL_PROFILE", "0"))),
    )
    _CACHE["last_exec_ns"] = res.exec_time_ns

    out = np.zeros((2, SEQ, D), dtype=np.float32)
    for c in range(8):
        out[c // 4] += res.results[c]["out"]
    return out


# revision 42
# speedup vs baseline: 1.0273x; 1.0273x over previous
"""Multi-head attention (B=2, S=2048, D=768, H=12) on 8 TRN2 NeuronCores.

Sharding: core c -> batch b = c//4, head-group g = c%4 (3 heads of 64 each).

v2 design (vs v1 baseline):
  - Softmax pipeline: E^T matmuls (K=64, h0/h1 row-group paired at partitions
    0-63/64-127; h2 paired with itself via duplicated k/q rows) -> scalar-engine
    exp (the only exp-capable engine; it paces the kernel at ~96us) -> 0/1 mask
    multiply in bf16 SBUF split across DVE (h0/h1) and GpSimd (h2) -> PV with a
    ones column in v for the denominator row.
  - v projected directly into [seq, d] layout (x-stationary matmuls), killing
    the 32 PE transposes of v1.
  - Output projection stacks h0/h1 into one K=128 matmul; h2 (K=64) pairs
    across q-tile parity via duplicated onorm2/wo2 rows.
  - Denominator reciprocal via reciprocal_approx_fast (0.7us vs 4us each).
  - Issue-order software pipeline: background projection / PV work drained
    between E columns to keep PE warm and dense.
"""

import os
import sys

sys.path.insert(0, "/opt/trn_rl_repo")

from collections import deque
from contextlib import ExitStack

import ml_dtypes
import numpy as np

import concourse.bass as bass
import concourse.mybir as mybir
import concourse.tile as tile
from concourse import bacc
from concourse.bass import ds
from concourse.bass_utils import run_bass_kernel_spmd

F32 = mybir.dt.float32
BF16 = mybir.dt.bfloat16

SEQ = 2048
D = 768
HD = 64
GD = 192          # head-group width = 3 heads * 64
QB = 512          # q-block
NQB = SEQ // QB   # 4
KT = SEQ // 128   # 16 k-tiles
NCH = 8           # chunks per (h, n): 2 k-tiles each
SCALE = float(1.0 / np.sqrt(np.float32(D)))

_CACHE = {}
DBG_EXACT_RECIP = bool(int(os.environ.get("DBG_EXACT_RECIP", "0")))
DBG_NO_H2DUP = bool(int(os.environ.get("DBG_NO_H2DUP", "0")))
DBG_ALL_VEC_MASK = bool(int(os.environ.get("DBG_ALL_VEC_MASK", "0")))
DBG_NO_COLPAIR = bool(int(os.environ.get("DBG_NO_COLPAIR", "0")))


def _install_profile_hook():
    import types

    if "antenv.axon_hooks" in sys.modules:
        return
    sys.path.insert(0, "/root/.axon_site")
    try:
        from trn_agent_boot.trn_boot import _ntff_profile_via_ctypes
        hook = _ntff_profile_via_ctypes("/opt/axon/libaxon_pjrt.so")
    except Exception:
        hook = None
    import concourse.bass_utils as _bu

    _bu.upload_artifacts = lambda tmpdir: tmpdir
    mod = types.ModuleType("antenv.axon_hooks")
    mod.get_axon_ntff_profile_hook = lambda: hook
    mod.set_axon_ntff_profile_hook = lambda h: None
    sys.modules["antenv.axon_hooks"] = mod


def _build():
    nc = bacc.Bacc(None)

    xqT = nc.declare_dram_parameter("xqT", [D, SEQ], BF16, isOutput=False)
    xkT = nc.declare_dram_parameter("xkT", [D, SEQ], BF16, isOutput=False)
    xvT = nc.declare_dram_parameter("xvT", [D, SEQ], BF16, isOutput=False)
    wqT = nc.declare_dram_parameter("wqT", [D, GD], BF16, isOutput=False)
    wkT = nc.declare_dram_parameter("wkT", [D, GD], BF16, isOutput=False)
    wvT = nc.declare_dram_parameter("wvT", [D, GD], BF16, isOutput=False)
    woT = nc.declare_dram_parameter("woT", [GD, D], BF16, isOutput=False)
    maskT = nc.declare_dram_parameter("maskT", [SEQ, SEQ], BF16, isOutput=False)
    out = nc.declare_dram_parameter("out", [SEQ, D], F32, isOutput=True)
    DBG_DUMP = bool(int(os.environ.get("DBG_DUMP", "0")))
    if DBG_DUMP:
        dbg = {}
        for nm in ("qA", "kA", "qB", "kB", "onormA", "onorm2"):
            dbg[nm] = nc.declare_dram_parameter(f"dbg_{nm}", [128, SEQ], BF16, isOutput=True)
        dbg["vall"] = nc.declare_dram_parameter("dbg_vall", [128, KT * 3 * 66], BF16, isOutput=True)
        dbg["P0"] = nc.declare_dram_parameter("dbg_P0", [128, KT * QB], BF16, isOutput=True)
        dbg["ou"] = nc.declare_dram_parameter("dbg_ou", [128, QB], F32, isOutput=True)

    with tile.TileContext(nc) as tc, ExitStack() as ctx:
        Exp = mybir.ActivationFunctionType.Exp

        # ---- persistent SBUF ------------------------------------------------
        pp = ctx.enter_context(tc.tile_pool(name="persist", bufs=1))
        qA = pp.tile([128, SEQ], BF16, tag="qA")    # h0 p0-63, h1 p64-127
        kA = pp.tile([128, SEQ], BF16, tag="kA")
        qB = pp.tile([128, SEQ], BF16, tag="qB")    # h2 duplicated both halves
        kB = pp.tile([128, SEQ], BF16, tag="kB")
        # v with ones column at 64 (66-wide for 4B alignment); [k, kt, h, 66]
        v_all = pp.tile([128, KT, 3, 66], BF16, tag="vall")
        onormA = pp.tile([128, SEQ], BF16, tag="onormA")  # h0 p0-63, h1 p64-127
        onorm2 = pp.tile([128, SEQ], BF16, tag="onorm2")  # h2 duplicated
        woA_sb = pp.tile([128, D], BF16, tag="woA")
        wo2_sb = pp.tile([128, D], BF16, tag="wo2")       # h2 rows duplicated
        w_sb = {}
        for name, wT in (("q", wqT), ("k", wkT), ("v", wvT)):
            w_sb[name] = pp.tile([128, 6, GD], BF16, tag=f"w{name}", name=f"w_{name}")
            nc.sync.dma_start(
                w_sb[name][:], wT.rearrange("(ko ki) d -> ki ko d", ki=128)
            )
        nc.sync.dma_start(woA_sb[:], woT[ds(0, 128), :])
        nc.sync.dma_start(wo2_sb[0:64, :], woT[ds(128, 64), :])
        nc.sync.dma_start(wo2_sb[64:128, :], woT[ds(128, 64), :])
        nc.vector.memset(v_all[:, :, :, 64:65], 1.0)

        # ---- x / mask staging ----------------------------------------------
        xvp = ctx.enter_context(tc.tile_pool(name="xv", bufs=1))
        xqp = ctx.enter_context(tc.tile_pool(name="xq", bufs=4))
        mp = ctx.enter_context(tc.tile_pool(name="mask", bufs=2))

        xv_sb = xvp.tile([128, 6, SEQ], BF16, tag="xv")
        xkp_stack = ExitStack()
        xkp = xkp_stack.enter_context(tc.tile_pool(name="xk", bufs=1))
        xk_sb = xkp.tile([128, 6, SEQ], BF16, tag="xk")
        xkTr = xkT.rearrange("(ko ki) s -> ki ko s", ki=128)
        xvTr = xvT.rearrange("(ko ki) s -> ki ko s", ki=128)
        xqTr = xqT.rearrange("(ko ki) s -> ki ko s", ki=128)
        maskTr = maskT.rearrange("(ko ki) q -> ki ko q", ki=128)
        for k in range(6):
            nc.sync.dma_start(xk_sb[:, k, :], xkTr[:, k, :])
        for k2 in range(3):
            nc.sync.dma_start(
                xv_sb[:, ds(k2 * 2, 2), :], xvTr[:, ds(k2 * 2, 2), :]
            )

        def dma_xq(n):
            t = xqp.tile([128, 6, QB], BF16, tag="xq", name=f"xq{n}")
            nc.sync.dma_start(t[:], xqTr[:, :, ds(n * QB, QB)])
            return t

        def dma_mask(n, half):
            t = mp.tile([128, NCH, QB], BF16, tag="m", name=f"m{n}_{half}")
            nc.sync.dma_start(
                t[:], maskTr[:, ds(half * NCH, NCH), ds(n * QB, QB)]
            )
            return t

        xq_t = {0: dma_xq(0)}
        mask_t = {(0, 0): dma_mask(0, 0), (0, 1): dma_mask(0, 1)}

        # ---- preamble projections: kA, B(n0), qA(n0) -----------------------
        pre_ps = ExitStack()
        pj = pre_ps.enter_context(tc.tile_pool(name="pj", bufs=4, space="PSUM"))
        psB = pre_ps.enter_context(tc.tile_pool(name="psB", bufs=1, space="PSUM"))

        def proj_A(name, x_ap, dst, n):
            """[128,512] psum accumulation of heads 0/1 for q-block n."""
            t = pj.tile([128, QB], F32, tag="pj", name=f"pj_{name}{n}")
            for k in range(6):
                nc.tensor.matmul(
                    t[:], lhsT=w_sb[name][:, k, ds(0, 128)],
                    rhs=x_ap[k], start=(k == 0), stop=(k == 5),
                )
            nc.vector.tensor_copy(dst[:, ds(n * QB, QB)], t[:])

        def proj_B(x_q, x_k, n, psum_tile=None):
            """col-paired h2 proj of q (rows 0-63) and k (rows 64-127)."""
            t = psum_tile if psum_tile is not None else psB.tile(
                [128, QB], F32, tag="psB", name=f"psB{n}"
            )
            if DBG_NO_COLPAIR:
                for k in range(6):
                    nc.tensor.matmul(
                        t[0:64, :], lhsT=w_sb["q"][:, k, ds(128, 64)],
                        rhs=x_q[k], start=(k == 0), stop=(k == 5),
                    )
                for k in range(6):
                    nc.tensor.matmul(
                        t[64:128, :], lhsT=w_sb["k"][:, k, ds(128, 64)],
                        rhs=x_k[k], start=(k == 0), stop=(k == 5),
                        tile_position=(0, 64),
                    )
            else:
                for k in range(6):
                    nc.tensor.matmul(
                        t[0:64, :], lhsT=w_sb["q"][:, k, ds(128, 64)],
                        rhs=x_q[k], start=(k == 0), stop=(k == 5),
                        tile_position=(0, 0),
                    )
                    nc.tensor.matmul(
                        t[64:128, :], lhsT=w_sb["k"][:, k, ds(128, 64)],
                        rhs=x_k[k], start=(k == 0), stop=(k == 5),
                        tile_position=(0, 64),
                    )
            sl = ds(n * QB, QB)
            nc.vector.tensor_copy(qB[0:64, sl], t[0:64, :])
            nc.vector.tensor_copy(qB[64:128, sl], t[0:64, :])
            nc.vector.tensor_copy(kB[0:64, sl], t[64:128, :])
            nc.vector.tensor_copy(kB[64:128, sl], t[64:128, :])

        # kA first (gates all E matmuls), then h2 B-pair n0, then qA n0
        kA_ps = [pj.tile([128, QB], F32, tag="pj", name=f"pj_k{n}") for n in range(NQB)]
        for k in range(6):
            for n in range(NQB):
                nc.tensor.matmul(
                    kA_ps[n][:], lhsT=w_sb["k"][:, k, ds(0, 128)],
                    rhs=xk_sb[:, k, ds(n * QB, QB)],
                    start=(k == 0), stop=(k == 5),
                )
        for n in range(NQB):
            nc.vector.tensor_copy(kA[:, ds(n * QB, QB)], kA_ps[n][:])
        for bn in range(NQB):
            if bn > 0 and bn not in xq_t:
                xq_t[bn] = dma_xq(bn)
            proj_B([xq_t[bn][:, k, :] for k in range(6)],
                   [xk_sb[:, k, ds(bn * QB, QB)] for k in range(6)], bn)
        proj_A("q", [xq_t[0][:, k, :] for k in range(6)], qA, 0)
        pre_ps.close()
        xkp_stack.close()

        # ---- attention pipeline --------------------------------------------
        att_ps = ExitStack()
        ou_ps = att_ps.enter_context(tc.tile_pool(name="ou", bufs=2, space="PSUM"))
        e_ps = att_ps.enter_context(tc.tile_pool(name="e", bufs=3, space="PSUM"))
        pp2 = ctx.enter_context(tc.tile_pool(name="P", bufs=3 if DBG_DUMP else 4))
        rp = ctx.enter_context(tc.tile_pool(name="rp", bufs=1))
        op = ctx.enter_context(tc.tile_pool(name="op", bufs=2))

        filler = deque()
        if DBG_DUMP:
            dbg_P0_sb = pp.tile([128, KT * QB], BF16, tag="dbgP0")
            dbg_ou_sb = pp.tile([128, QB], F32, tag="dbgou")

        def v_pair_task(s):
            def run():
                t = ou_ps.tile([128, GD], F32, tag="ou", name=f"vps{s}")
                for k in range(6):
                    nc.tensor.matmul(
                        t[:], lhsT=xv_sb[:, k, ds(s * 128, 128)],
                        rhs=w_sb["v"][:, k, :],
                        start=(k == 0), stop=(k == 5),
                    )
                nc.vector.tensor_copy(v_all[:, s, :, 0:64], t[:])
            return run

        def qA_task(n):
            def run():
                t = e_ps.tile([128, 2, QB], F32, tag="e", name=f"qa{n}")
                for k in range(6):
                    nc.tensor.matmul(
                        t[:, 0, :], lhsT=w_sb["q"][:, k, ds(0, 128)],
                        rhs=xq_t[n][:, k, :], start=(k == 0), stop=(k == 5),
                    )
                nc.vector.tensor_copy(qA[:, ds(n * QB, QB)], t[:, 0, :])
            return run

        def qB_task(n):
            def run():
                t = e_ps.tile([128, 2, QB], F32, tag="e", name=f"qb{n}")
                proj_B([xq_t[n][:, k, :] for k in range(6)],
                       [xk_sb[:, k, ds(n * QB, QB)] for k in range(6)],
                       n, psum_tile=t[:, 0, :])
            return run

        def pv_task(h, n, P_t, grp):
            def run():
                if grp == 0:
                    pv_task.ou[(h, n)] = ou_ps.tile(
                        [HD + 1, QB], F32, tag="ou", name=f"ou{h}_{n}"
                    )
                t = pv_task.ou[(h, n)]
                for j in range(4):
                    m = grp * 4 + j
                    nc.tensor.matmul(
                        t[:], lhsT=v_all[:, m, h, 0:65],
                        rhs=P_t[:, m, :],
                        start=(m == 0), stop=(m == KT - 1),
                    )
                if grp == 3:
                    # normalize: recip of denominator row, broadcast, multiply
                    if DBG_DUMP and h == 2 and n == 0:
                        nc.vector.tensor_copy(dbg_ou_sb[0:65, :], t[:])
                    sl = ds(n * QB, QB)
                    d_sb = rp.tile([1, QB], F32, tag="dc", name=f"dc{h}_{n}")
                    nc.vector.tensor_copy(d_sb[:], t[HD : HD + 1, :])
                    r1 = rp.tile([1, QB], F32, tag="r1", name=f"r1{h}_{n}")
                    if DBG_EXACT_RECIP:
                        nc.vector.reciprocal(r1[:], d_sb[:])
                    else:
                        nc.vector.reciprocal_approx_fast(r1[:], d_sb[:])
                    rb = rp.tile([64, QB], F32, tag="rb", name=f"rb{h}_{n}")
                    nc.gpsimd.partition_broadcast(rb[:], r1[:])
                    if DBG_DUMP and h == 2 and n == 0:
                        nc.vector.tensor_copy(dbg_ou_sb[96:97, :], r1[:])
                    if h == 0:
                        nc.vector.tensor_mul(onormA[0:64, sl], t[0:HD, :], rb[:])
                    elif h == 1:
                        nc.vector.tensor_mul(onormA[64:128, sl], t[0:HD, :], rb[:])
                    else:
                        nc.vector.tensor_mul(onorm2[0:64, sl], t[0:HD, :], rb[:])
                        nc.vector.tensor_mul(onorm2[64:128, sl], t[0:HD, :], rb[:])
            return run

        pv_task.ou = {}

        def oproj_task(mq):
            def run():
                p0 = 0 if mq % 2 == 0 else 64
                o_sb = op.tile([128, D], F32, tag="o", name=f"o{mq}")
                for j, (n0, nw) in enumerate(((0, 512), (512, 256))):
                    fp = ou_ps.tile([128, nw], F32, tag="ou", name=f"f{mq}_{j}")
                    nc.tensor.matmul(
                        fp[:], lhsT=onormA[:, ds(mq * 128, 128)],
                        rhs=woA_sb[:, ds(n0, nw)], start=True, stop=False,
                    )
                    nc.tensor.matmul(
                        fp[:], lhsT=onorm2[p0 : p0 + 64, ds(mq * 128, 128)],
                        rhs=wo2_sb[p0 : p0 + 64, ds(n0, nw)],
                        start=False, stop=True,
                    )
                    eng = nc.scalar if mq % 2 == 0 else nc.vector
                    if mq % 2 == 0:
                        nc.scalar.copy(o_sb[:, ds(n0, nw)], fp[:])
                    else:
                        nc.vector.tensor_copy(o_sb[:, ds(n0, nw)], fp[:])
                d_eng = (nc.sync, nc.gpsimd)[mq % 2]
                d_eng.dma_start(out[ds(mq * 128, 128), :], o_sb[:])
            return run

        def drain(k):
            for _ in range(k):
                if filler:
                    filler.popleft()()

        for s in range(KT):
            filler.append(v_pair_task(s))

        q_of = (qA, qA, qB)
        k_of = (kA, kA, kB)
        pbase = (0, 64, 0)

        for n in range(NQB):

            P = [pp2.tile([128, KT, QB], BF16, tag="P", name=f"P{n}_{h}")
                 for h in range(3)]
            if DBG_DUMP and n == 1:
                nc.vector.tensor_copy(dbg_P0_sb[:], P_prev0[:])
            if n == 0:
                P_prev0 = P[2]
            nsl = ds(n * QB, QB)
            for c in range(NCH):
                m0, m1 = 2 * c, 2 * c + 1
                half, hsl = c // 4, ds((c % 4) * 2, 2)
                mk = mask_t[(n, half)]
                for h in range(3):
                    e = e_ps.tile([128, 2, QB], F32, tag="e", name=f"e{n}_{c}_{h}")
                    for mm, m in ((0, m0), (1, m1)):
                        if h == 2:
                            p0 = 0 if (mm == 0 or DBG_NO_H2DUP) else 64
                        else:
                            p0 = pbase[h]
                        nc.tensor.matmul(
                            e[:, mm, :],
                            lhsT=k_of[h][p0 : p0 + 64, ds(m * 128, 128)],
                            rhs=q_of[h][p0 : p0 + 64, nsl],
                            start=True, stop=True,
                        )
                    if DBG_DUMP and n == 0 and c == 4 and h == 2:
                        nc.vector.tensor_copy(dbg_ou_sb[0:128, :], e[:, 0, :])
                    nc.scalar.activation(
                        P[h][:, ds(2 * c, 2), :], e[:, :, :], Exp,
                        scale=SCALE,
                    )
                # batched mask multiplies: DVE for h0/h1 every 2 chunks,
                # gpsimd for h2 once per mask half (big op, fewer dispatches)
                if c % 2 == 1:
                    for h in (0, 1):
                        nc.vector.tensor_mul(
                            P[h][:, ds((c - 1) * 2, 4), :],
                            P[h][:, ds((c - 1) * 2, 4), :],
                            mk[:, ds(((c - 1) % 4) * 2, 4), :],
                        )
                if c % 4 == 3:
                    eng2 = nc.vector if DBG_ALL_VEC_MASK else nc.gpsimd
                    eng2.tensor_mul(
                        P[2][:, ds((c - 3) * 2, 8), :],
                        P[2][:, ds((c - 3) * 2, 8), :],
                        mk[:, :, :],
                    )
                drain(3)
                if c == 3 and n + 1 < NQB:
                    mask_t[(n + 1, 0)] = dma_mask(n + 1, 0)
                if c == 5 and n + 1 < NQB:
                    qA_task(n + 1)()
            if n + 1 < NQB:
                mask_t[(n + 1, 1)] = dma_mask(n + 1, 1)
            for h in range(3):
                for g in range(4):
                    filler.append(pv_task(h, n, P[h], g))
            for mq in range(4 * n, 4 * n + 4):
                filler.append(oproj_task(mq))
            drain(6)

        while filler:
            drain(1)
        att_ps.close()
    nc.compile()
    return nc


def kernel(Q, K, V, mask, Wq, Wk, Wv, Wo):
    if "nc" not in _CACHE:
        _CACHE["nc"] = _build()
    nc = _CACHE["nc"]

    maskT_bf = np.ascontiguousarray(
        (mask[0, 0].T != 0).astype(ml_dtypes.bfloat16)
    )
    in_maps = []
    for c in range(8):
        b, g = c // 4, c % 4
        sl = slice(g * GD, (g + 1) * GD)
        in_maps.append(
            {
                "xqT": np.ascontiguousarray(Q[b].T.astype(ml_dtypes.bfloat16)),
                "xkT": np.ascontiguousarray(K[b].T.astype(ml_dtypes.bfloat16)),
                "xvT": np.ascontiguousarray(V[b].T.astype(ml_dtypes.bfloat16)),
                "wqT": np.ascontiguousarray(Wq[sl, :].T.astype(ml_dtypes.bfloat16)),
                "wkT": np.ascontiguousarray(Wk[sl, :].T.astype(ml_dtypes.bfloat16)),
                "wvT": np.ascontiguousarray(Wv[sl, :].T.astype(ml_dtypes.bfloat16)),
                "woT": np.ascontiguousarray(Wo[:, sl].T.astype(ml_dtypes.bfloat16)),
                "maskT": maskT_bf,
            }
        )

    _install_profile_hook()
    res = run_bass_kernel_spmd(
        nc,
        in_maps,
        core_ids=list(range(8)),
        trace=bool(int(os.environ.get("KERNEL_PROFILE", "0"))),
    )
    _CACHE["last_exec_ns"] = res.exec_time_ns

    out = np.zeros((2, SEQ, D), dtype=np.float32)
    for c in range(8):
        out[c // 4] += res.results[c]["out"]
    return out
